# revision 8
# baseline (speedup 1.0000x reference)
"""GWPooling2D forward on 8 Trainium2 NeuronCores.

y[b, c, o] = sum_k m[c, o, k] * x[b, k]   (k = 400 input pixels, o = 256)

The pooling map m depends only on the small `signal` parameter and is
computed on host exactly as in the reference. It decomposes as

    m[c] = P0 + E[c]

where P0 (256 x 400) is the signal-independent resampling map (expm(0)=I
pushed through the same crop/roll/FFT pipeline) shared by all 16 channels,
and E[c] is the small per-channel correction (||E|| ~ 0.17 ||m||).

Device work per core (1024-batch shard, data parallel across 8 cores):
  yP = x_bf16 @ P0_bf16^T            (bf16 matmuls, 256 cols)
  yE = x_fp8  @ (E * s)_fp8^T        (fp8e4m3 DoubleRow matmuls, 4096 cols,
                                      K=400 in 2 packed chunks of 2x128/2x72)
yE is written back as fp8 (it is ~17% of y, so fp8 noise is ~0.6% of y),
yP as bf16; the host computes y = yP + yE/s. The single scale s keeps both
the quantized E and the yE PSUM values inside fp8e4m3 range (+-240).
"""

import numpy as np
import scipy.linalg

import concourse.bass as bass
import concourse.bacc as bacc
import concourse.mybir as mybir
import concourse.tile as tile
from concourse.bass_utils import run_bass_kernel_spmd
import ml_dtypes

C = 16
P = (24, 24)
NI = (20, 20)
NO = (16, 16)
B = 8192
NCORES = 8
BS = B // NCORES              # 1024 batch rows per core
K = NI[0] * NI[1]             # 400 contraction
O = NO[0] * NO[1]             # 256 output positions per channel
CO = C * O                    # 4096 (c,o) output columns
BT = 128                      # batch tile (PSUM partitions)
OT = 512                      # output-feature tile (PSUM free dim)
K0 = 256                      # DoubleRow chunk 0 (2 x 128 partitions)
K1 = K - K0                   # 144 = 2 x 72 partitions

F8 = ml_dtypes.float8_e4m3
BF16 = ml_dtypes.bfloat16


# ---------------------------------------------------------------- host map ---

def _hann(n):
    return 0.5 * (1.0 - np.cos(2.0 * np.pi * np.arange(n) / n))


def _signal_to_spectrum(signal):
    n0, n1 = signal.shape[-2], signal.shape[-1]
    window = _hann(n0)[:, None] * _hann(n1)[None, :]
    rx = np.arange((-n0) // 2 + 1, n0 // 2 + 1)[:, None]
    ry = np.arange((-n1) // 2 + 1, n1 // 2 + 1)[None, :]
    r = (1 + rx * rx + ry * ry).astype(np.float64)
    wf = np.roll(np.fft.fft2(signal), (n0 // 2, n1 // 2), (-2, -1)) / r / 5.0
    wt = np.fft.ifft2(np.roll(wf, (-(n0 // 2), -(n1 // 2)), (-2, -1))) * window
    return np.roll(np.fft.fft2(wt), (n0 // 2, n1 // 2), (-2, -1))


def _gw2d_algebra(w):
    p0, p1 = w.shape[-2], w.shape[-1]
    pad = [(0, 0)] * (w.ndim - 2) + [(p1 // 2, p1 // 2), (p0 // 2, p0 // 2)]
    wp = np.pad(w, pad)
    ia = np.arange(p0)[:, None] + np.arange(p0)[None, :]
    jb = np.arange(p1)[:, None] + np.arange(p1)[None, :]
    ws = wp[..., ia[:, None, :, None], jb[None, :, None, :]]
    ws = ws[..., ::-1, ::-1, :, :]
    kx = np.arange((-p0) // 2 + 1, p0 // 2 + 1)[:, None]
    ky = np.arange((-p1) // 2 + 1, p1 // 2 + 1)[None, :]
    return -1j * (ws[..., 0, :, :, :, :] * kx + ws[..., 1, :, :, :, :] * ky)


def _transform_to_map(t):
    p0, p1 = t.shape[-2], t.shape[-1]
    di = (p0 - NI[0], p1 - NI[1])
    do = (p0 - NO[0], p1 - NO[1])
    x = t[..., do[0] // 2 + 1:(-do[0]) // 2 + 1, do[1] // 2 + 1:(-do[1]) // 2 + 1,
          di[0] // 2 + 1:(-di[0]) // 2 + 1, di[1] // 2 + 1:(-di[1]) // 2 + 1]
    x = np.roll(x, (NO[0] // 2 + 1, NO[1] // 2 + 1, NI[0] // 2 + 1, NI[1] // 2 + 1),
                (-4, -3, -2, -1))
    return np.fft.fft2(np.fft.ifft2(x, axes=(-2, -1)), axes=(-4, -3)).real


def compute_mf(signal):
    """signal (C,2,24,24) -> pooling matrix (CO=4096, K=400) float32."""
    spectrum = _signal_to_spectrum(signal.astype(np.float64))
    p0, p1 = spectrum.shape[-2], spectrum.shape[-1]
    a = _gw2d_algebra(spectrum)
    n = p0 * p1
    mat = a.reshape(a.shape[:-4] + (n, n))
    t = np.stack([scipy.linalg.expm(mat[i]) for i in range(mat.shape[0])])
    t = t.reshape(t.shape[:-2] + (p0, p1, p0, p1))
    m = _transform_to_map(t)
    return m.reshape(CO, K).astype(np.float32)


_P0 = None


def compute_p0():
    """Signal-independent part of the map: expm(0)=I through the same
    crop/roll/FFT pipeline. (256, 400) float64."""
    global _P0
    if _P0 is None:
        t_id = np.eye(P[0] * P[1], dtype=np.complex128).reshape(
            1, P[0], P[1], P[0], P[1])
        _P0 = _transform_to_map(t_id).reshape(O, K)
    return _P0


# ------------------------------------------------------------ device kernel ---

_built = None


def _build():
    global _built
    if _built is not None:
        return _built
    nc = bacc.Bacc(dynamic_dma_scratch_size=16384)
    f32 = mybir.dt.float32
    bf16 = mybir.dt.bfloat16
    f8 = mybir.dt.float8e4
    DR = mybir.MatmulPerfMode.DoubleRow

    xb_d = nc.declare_dram_parameter("xb", (K, BS), bf16, isOutput=False)
    p16_d = nc.declare_dram_parameter("p16", (K, O), bf16, isOutput=False)
    x80_d = nc.declare_dram_parameter("x80", (K0 // 2, 2, BS), f8, isOutput=False)
    x81_d = nc.declare_dram_parameter("x81", (K1 // 2, 2, BS), f8, isOutput=False)
    e80_d = nc.declare_dram_parameter("e80", (K0 // 2, 2, CO), f8, isOutput=False)
    e81_d = nc.declare_dram_parameter("e81", (K1 // 2, 2, CO), f8, isOutput=False)
    outE_d = nc.declare_dram_parameter("outE", (BS, CO), f8, isOutput=True)
    outP_d = nc.declare_dram_parameter("outP", (BS, O), bf16, isOutput=True)

    NB = BS // BT                 # 8 batch tiles
    NCO = CO // OT                # 8 E-column tiles
    G = 4                         # co-tiles per staged store
    KP = 100                      # bf16 path contraction chunk

    with tile.TileContext(nc) as tc:
        with tc.tile_pool(name="inpool", bufs=1) as inpool, \
             tc.tile_pool(name="opool", bufs=4) as opool, \
             tc.tile_pool(name="pepool", bufs=4, space="PSUM") as pepool:
            # ---- loads, ordered so the first DoubleRow matmul can start
            # as early as possible ----
            NEQ = 4
            EQ = CO // NEQ                      # 1024 columns per quarter
            e80q, e81q = [None] * NEQ, [None] * NEQ

            def load_equarter(q):
                t0 = inpool.tile([K0 // 2, 2, EQ], f8, tag=f"e80q{q}",
                                 name=f"e80q{q}")
                nc.sync.dma_start(t0[:], e80_d[:, :, q * EQ:(q + 1) * EQ])
                e80q[q] = t0
                t1 = inpool.tile([K1 // 2, 2, EQ], f8, tag=f"e81q{q}",
                                 name=f"e81q{q}")
                nc.sync.dma_start(t1[:], e81_d[:, :, q * EQ:(q + 1) * EQ])
                e81q[q] = t1

            x80 = inpool.tile([K0 // 2, 2, BS], f8, name="x80")
            nc.sync.dma_start(x80[:], x80_d[:])
            load_equarter(0)
            x81 = inpool.tile([K1 // 2, 2, BS], f8, name="x81")
            nc.sync.dma_start(x81[:], x81_d[:])
            load_equarter(1)
            xb = inpool.tile([KP, K // KP, BS], bf16, name="xb")
            nc.sync.dma_start(xb[:], xb_d.rearrange("(c p) b -> p c b", p=KP))
            p16 = inpool.tile([KP, K // KP, O], bf16, name="p16")
            nc.sync.dma_start(p16[:], p16_d.rearrange("(c p) o -> p c o", p=KP))
            load_equarter(2)
            load_equarter(3)

            ncopy = 0

            def cast_copy(dst, src):
                nonlocal ncopy
                # ACT is a bit faster than DVE: give it 5 of every 9
                eng = (nc.vector.tensor_copy, nc.scalar.copy,
                       nc.scalar.copy, nc.vector.tensor_copy,
                       nc.scalar.copy, nc.vector.tensor_copy,
                       nc.scalar.copy, nc.vector.tensor_copy,
                       nc.scalar.copy)[ncopy % 9]
                eng(dst, src)
                ncopy += 1

            def e_group(b, cp):
                # one staging tile = 4 co-tiles = 2 double-bank PSUM tiles
                st = opool.tile([BT, G * OT], f8, name="st")
                for h in range(2):
                    ps = pepool.tile([BT, 2 * OT], f32, name="ps")
                    for j2 in range(2):
                        co = cp * G + h * 2 + j2
                        q, cof = divmod(co * OT, EQ)
                        nc.tensor.matmul(
                            ps[:, j2 * OT:(j2 + 1) * OT],
                            x80[:, :, b * BT:(b + 1) * BT],
                            e80q[q][:, :, cof:cof + OT],
                            start=True, stop=False, perf_mode=DR,
                        )
                        nc.tensor.matmul(
                            ps[:, j2 * OT:(j2 + 1) * OT],
                            x81[:, :, b * BT:(b + 1) * BT],
                            e81q[q][:, :, cof:cof + OT],
                            start=False, stop=True, perf_mode=DR,
                        )
                    cast_copy(st[:, h * 2 * OT:(h + 1) * 2 * OT], ps[:])
                nc.gpsimd.dma_start(
                    outE_d[b * BT:(b + 1) * BT, cp * G * OT:(cp + 1) * G * OT],
                    st[:])

            # ---- E columns 0..2047 ----
            for b in range(NB):
                e_group(b, 0)

            # ---- P part: yP = x_bf16 @ P0^T (PSUM reused from the E pool) ----
            yps = opool.tile([BT, NB, O], bf16, tag="yps", name="yps")
            for b in range(NB):
                pp = pepool.tile([BT, 2 * OT], f32, name="ps")
                for ci in range(K // KP):
                    nc.tensor.matmul(
                        pp[:, :O],
                        xb[:, ci, b * BT:(b + 1) * BT],
                        p16[:, ci, :],
                        start=(ci == 0),
                        stop=(ci == K // KP - 1),
                    )
                cast_copy(yps[:, b, :], pp[:, :O])
            nc.gpsimd.dma_start(outP_d.rearrange("(j p) o -> p j o", p=BT), yps[:])

            # ---- E columns 2048..4095 ----
            for b in range(NB):
                e_group(b, 1)
    nc.compile()
    _built = nc
    return nc


def _prep_host(x, signal):
    """Host-side factorization + quantization. Returns per-core input maps
    and the dequantization scale."""
    mf = compute_mf(np.asarray(signal))                     # (4096, 400)
    p0 = compute_p0()                                       # (256, 400) f64
    e = mf.astype(np.float64).reshape(C, O, K) - p0[None]
    ef = e.reshape(CO, K)

    # single scale: keeps E*s inside fp8 range and (with 8-sigma slack for
    # x ~ N(0,1)) the yE accumulator inside +-240 at the fp8 store
    row_norm = np.sqrt((ef * ef).sum(axis=1)).max()
    s = min(200.0 / np.abs(ef).max(), 200.0 / (8.0 * row_norm))
    e8 = (ef * s).astype(np.float32).astype(F8)             # (4096, 400)

    # DoubleRow packing: chunk0 k = i*128 + ki, chunk1 k = 256 + i*72 + ki
    e8c0 = np.ascontiguousarray(
        e8[:, :K0].reshape(CO, 2, K0 // 2).transpose(2, 1, 0))   # (128,2,4096)
    e8c1 = np.ascontiguousarray(
        e8[:, K0:].reshape(CO, 2, K1 // 2).transpose(2, 1, 0))   # (72,2,4096)

    p16 = np.ascontiguousarray(p0.T.astype(np.float32).astype(BF16))  # (400,256)

    xT = np.asarray(x).reshape(B, K).T                      # (400, 8192) f32
    xTb = xT.astype(BF16)
    x8 = xT.astype(F8)
    x8c0 = x8[:K0].reshape(2, K0 // 2, B).transpose(1, 0, 2)      # (128,2,8192)
    x8c1 = x8[K0:].reshape(2, K1 // 2, B).transpose(1, 0, 2)      # (72,2,8192)

    in_maps = []
    for i in range(NCORES):
        bs = slice(i * BS, (i + 1) * BS)
        in_maps.append({
            "xb": np.ascontiguousarray(xTb[:, bs]),
            "p16": p16,
            "x80": np.ascontiguousarray(x8c0[:, :, bs]),
            "x81": np.ascontiguousarray(x8c1[:, :, bs]),
            "e80": e8c0,
            "e81": e8c1,
        })
    return in_maps, s


def _run(x, signal, **spmd_kwargs):
    nc = _build()
    in_maps, s = _prep_host(x, signal)
    res = run_bass_kernel_spmd(nc, in_maps, list(range(NCORES)), **spmd_kwargs)
    parts = []
    for r in res.results:
        yE = r["outE"].astype(np.float32).reshape(BS, C, O) / s
        yP = r["outP"].astype(np.float32)
        parts.append(yE + yP[:, None, :])
    y = np.concatenate(parts, axis=0)
    return y.reshape(B, C, NO[0], NO[1]), res


def kernel(x, signal):
    y, _ = _run(x, signal)
    return y
